# revision 7
# baseline (speedup 1.0000x reference)
"""Trainium2 Bass kernel for nn_JointSupervisedGroundedCoreferencer.

Reference computation (B=8 docs, N=40 spans, L=20 images, D=H=1024):
  grounding: all-pairs MLP over (B*N) x (B*L) pairs:
      h  = relu(f@Wf + v@Ws + (f*v)@Wp + b1)   (decomposed 3D-concat linear)
      g  = relu(h@W2 + b2);  sc = g@W3 + b3    -> (BN, BL) scores
  text: per-doc span-pair MLP over triu pairs (P=780/doc)
  loss: bidirectional log-softmax CE on S = per-(doc,doc) summed grounding.

Sharding: core k owns doc k's 40 spans ("first" axis rows of the pair
matrix: 40x160 grounding pairs + 780 text pairs). MLP weights replicated.
Outputs gathered on host; the (tiny) log-softmax loss is computed on host
from the gathered score matrix exactly as the reference does.

All matmuls run on the PE in float32r (fp32 with mantissa rounded to 11
bits, full PE rate). Operands feeding matmuls are rounded to fp32r either
on host (weights, via zeroing the low 12 mantissa bits) or on device (DVE/
ACT outputs written with fp32r dtype).
"""
import numpy as np

import concourse.bass as bass
import concourse.tile as tile
from concourse import bacc, mybir
from concourse.bass import AP
from concourse.bass_utils import run_bass_kernel_spmd

f32 = mybir.dt.float32
f32r = mybir.dt.float32r
Alu = mybir.AluOpType
Act = mybir.ActivationFunctionType

D = 1024
H = 1024
B = 8
N = 40
L = 20
BL = B * L              # 160
NPG = N * BL            # 6400 grounding pairs per core
PT = N * (N - 1) // 2   # 780 text pairs per core
CH = 512                # pair-chunk size (one fp32 PSUM bank)
NT = 8                  # 128-tiles per 1024 dim

# text pair block offsets: pairs ordered (a, b) a<b, a-major
_TOFF = [0]
for _a in range(N - 1):
    _TOFF.append(_TOFF[-1] + (N - 1 - _a))


def _gsegments(start, size):
    """grounding chunk [start, start+size) split at i-boundaries (period BL).
    yields (i, j0, off_in_chunk, ln)."""
    segs = []
    pos, end = start, start + size
    while pos < end:
        i, j0 = divmod(pos, BL)
        ln = min(BL - j0, end - pos)
        segs.append((i, j0, pos - start, ln))
        pos += ln
    return segs


def _tsegments(start, size):
    """text chunk split at a-boundaries. yields (a, b0, off_in_chunk, ln)
    where b0 is the first second-span index of the segment."""
    segs = []
    end = start + size
    for a in range(N - 1):
        lo, hi = max(start, _TOFF[a]), min(end, _TOFF[a + 1])
        if lo < hi:
            segs.append((a, a + 1 + (lo - _TOFF[a]), lo - start, hi - lo))
    return segs


def _chunks(total):
    out = []
    pos = 0
    while pos < total:
        out.append((pos, min(CH, total - pos)))
        pos += CH
    return out


def round_fp32r(x):
    return (np.ascontiguousarray(x).view(np.uint32) & np.uint32(0xFFFFF000)).view(
        np.float32
    )


def _mm_group(nc, psum, lhsT_slices, rhs_slices):
    n = len(lhsT_slices)
    for k, (lh, rh) in enumerate(zip(lhsT_slices, rhs_slices)):
        nc.tensor.matmul(psum, lhsT=lh, rhs=rh, start=(k == 0), stop=(k == n - 1))


def _emit(tc, ctx):
    nc = tc.nc

    def din(name, shape, dt=f32):
        return nc.dram_tensor(name, shape, dt, kind="ExternalInput").ap()

    # per-core inputs (w* tensors are host-rounded fp32r)
    fT = din("fT", [D, N])                 # doc k spans, transposed
    vT = din("vT", [D, BL])                # all images, transposed
    gWf = din("gWf", [D, H], f32r)
    gWs = din("gWs", [D, H], f32r)
    gWp = din("gWp", [D, H], f32r)
    gW2 = din("gW2", [H, H], f32r)
    gW3 = din("gW3", [H], f32r)
    tW1a = din("tW1a", [D, H], f32r)
    tW1b = din("tW1b", [D, H], f32r)
    tW1c = din("tW1c", [D, H], f32r)
    tW2 = din("tW2", [H, H], f32r)
    tW3 = din("tW3", [H], f32r)
    gb1 = din("gb1", [H])
    gb2 = din("gb2", [H])
    gb3 = din("gb3", [1, 1])
    tb1 = din("tb1", [H])
    tb2 = din("tb2", [H])
    tb3 = din("tb3", [1, 1])
    mskg = din("mskg", [NPG])              # span_mask[k,i]*image_mask[j] pairs
    mskt = din("mskt", [PT])               # span_mask[k,fi]*span_mask[k,si]

    sc_out = nc.dram_tensor("sc_out", [NPG], f32, kind="ExternalOutput").ap()
    ts_out = nc.dram_tensor("ts_out", [PT], f32, kind="ExternalOutput").ap()

    wpool = ctx.enter_context(tc.tile_pool(name="w", bufs=3))
    perm = ctx.enter_context(tc.tile_pool(name="perm", bufs=1))
    vtp = ctx.enter_context(tc.tile_pool(name="vtp", bufs=2))
    htp = ctx.enter_context(tc.tile_pool(name="htp", bufs=1))
    gtp = ctx.enter_context(tc.tile_pool(name="gtp", bufs=1))
    scr = ctx.enter_context(tc.tile_pool(name="scr", bufs=2))
    mskp = ctx.enter_context(tc.tile_pool(name="mskp", bufs=2))
    outp = ctx.enter_context(tc.tile_pool(name="outp", bufs=2))
    psmm = ctx.enter_context(tc.tile_pool(name="psmm", bufs=4, space="PSUM"))
    psrow = ctx.enter_context(tc.tile_pool(name="psrow", bufs=2, space="PSUM"))
    psA = ctx.enter_context(tc.tile_pool(name="psA", bufs=2, space="PSUM"))

    dma = nc.sync.dma_start

    last_wdma = None

    def load_w(ap_in):
        """(1024, H) fp32r weight -> (128, 8, H) tile; serialize in call order."""
        nonlocal last_wdma
        t = wpool.tile([128, NT, H], f32r, tag="w")
        ins = dma(out=t, in_=ap_in.rearrange("(kt p) h -> p kt h", p=128))
        if last_wdma is not None:
            bass._add_dep_helper(ins.ins, last_wdma.ins, sync=True, reason="dma order")
        last_wdma = ins
        return t

    def col_tile(ap_in, dt=f32):
        """(1024,) -> (128, 8) per-partition column tile."""
        t = perm.tile([128, NT], dt, tag=ap_in.tensor.name)
        dma(out=t, in_=ap_in.rearrange("(t p) -> p t", p=128))
        return t

    # ---- small persistent loads --------------------------------------
    fTs = perm.tile([128, NT, N], f32, tag="fT")
    dma(out=fTs, in_=fT.rearrange("(dt p) i -> p dt i", p=128))
    vTs = perm.tile([128, NT, BL], f32, tag="vT")
    dma(out=vTs, in_=vT.rearrange("(dt p) j -> p dt j", p=128))
    # fp32r copies used as matmul rhs in the small (phase A) matmuls
    fTr = perm.tile([128, NT, N], f32r, tag="fTr")
    nc.vector.tensor_copy(fTr, fTs)
    gb1c = col_tile(gb1)
    gb2c = col_tile(gb2)
    tb1c = col_tile(tb1)
    tb2c = col_tile(tb2)
    gW3c = col_tile(gW3, f32r)
    tW3c = col_tile(tW3, f32r)
    gb3c = perm.tile([1, 1], f32, tag="gb3")
    dma(out=gb3c, in_=gb3)
    tb3c = perm.tile([1, 1], f32, tag="tb3")
    dma(out=tb3c, in_=tb3)

    # ---- weight order: grounding-first (phase B gates on gWp) --------
    gWp_t = load_w(gWp)

    gchunks = _chunks(NPG)
    tchunks = _chunks(PT)

    # ---- chunk 0 L1 emitted before phase A so PE starts on it ASAP ---
    def build_VT(csz, segs):
        VT = vtp.tile([128, NT, CH], f32r, tag="vt")
        for (i, j0, off, ln) in segs:
            for dt in range(NT):
                nc.vector.tensor_scalar_mul(
                    VT[:, dt, off : off + ln],
                    vTs[:, dt, j0 : j0 + ln],
                    fTs[:, dt, i : i + 1],
                )
        return VT

    def l1_mms(W1t, Xt, csz):
        psums = []
        for ht in range(NT):
            ps = psmm.tile([128, CH], f32, tag="mm")
            _mm_group(
                nc,
                ps[:, :csz],
                [W1t[:, dt, ht * 128 : (ht + 1) * 128] for dt in range(NT)],
                [Xt[:, dt, :csz] for dt in range(NT)],
            )
            psums.append(ps)
        return psums

    # ---- phase A-ground: aprimeT = f@Wf + b1 (cols), bT = v@Ws -------
    def small_matmuls(Wt, rhs, n, bias_col, out_tag):
        """out[h-part, n] tiles: (128, 8, n); bias_col None -> plain copy."""
        ot = perm.tile([128, NT, n], f32, tag=out_tag)
        for ht in range(NT):
            ps = psA.tile([128, n], f32, tag="small")
            _mm_group(
                nc,
                ps,
                [Wt[:, dt, ht * 128 : (ht + 1) * 128] for dt in range(NT)],
                [rhs[:, dt, :] for dt in range(NT)],
            )
            if bias_col is not None:
                nc.scalar.activation(
                    out=ot[:, ht, :], in_=ps, func=Act.Identity,
                    bias=bias_col[:, ht : ht + 1], scale=1.0,
                )
            else:
                nc.scalar.copy(ot[:, ht, :], ps)
        return ot

    # chunk 0 VT + L1 first (only needs gWp + fTs/vTs)
    c0_start, c0_sz = gchunks[0]
    c0_segs = _gsegments(c0_start, c0_sz)
    VT0 = build_VT(c0_sz, c0_segs)
    ps1_0 = l1_mms(gWp_t, VT0, c0_sz)

    gWf_t = load_w(gWf)
    gWs_t = load_w(gWs)
    vTr = perm.tile([128, NT, BL], f32r, tag="vTr")
    nc.vector.tensor_copy(vTr, vTs)
    aprimeT = small_matmuls(gWf_t, fTr, N, gb1c, "aprimeT")
    bT = small_matmuls(gWs_t, vTr, BL, None, "bT")

    gW2_t = load_w(gW2)
    tW1a_t = load_w(tW1a)
    tW1b_t = load_w(tW1b)

    # ---- main grounding pipeline -------------------------------------
    def epilogue_l1(psums, csz, segs, aT, bTt, per_a_text=False):
        """hT[:, ht, p] = relu(psum[ht] + a_col + b_row_slice)  (fp32r out)"""
        hT = htp.tile([128, NT, CH], f32r, tag="ht")
        for ht in range(NT):
            t = scr.tile([128, CH], f32, tag="scr")
            for (i, j0, off, ln) in segs:
                nc.vector.scalar_tensor_tensor(
                    out=t[:, off : off + ln],
                    in0=psums[ht][:, off : off + ln],
                    scalar=aT[:, ht, i : i + 1],
                    in1=bTt[:, ht, j0 : j0 + ln],
                    op0=Alu.add,
                    op1=Alu.add,
                )
            nc.scalar.activation(
                out=hT[:, ht, :csz], in_=t[:, :csz], func=Act.Relu,
                bias=0.0, scale=1.0,
            )
        return hT

    def l2_l3_out(hT, csz, W2t, b2c, W3col, b3c, msk_dram, out_dram, start):
        gT = gtp.tile([128, NT, CH], f32r, tag="gt")
        for h2t in range(NT):
            ps2 = psmm.tile([128, CH], f32, tag="mm")
            _mm_group(
                nc,
                ps2[:, :csz],
                [W2t[:, ht, h2t * 128 : (h2t + 1) * 128] for ht in range(NT)],
                [hT[:, ht, :csz] for ht in range(NT)],
            )
            nc.scalar.activation(
                out=gT[:, h2t, :csz], in_=ps2[:, :csz], func=Act.Relu,
                bias=b2c[:, h2t : h2t + 1], scale=1.0,
            )
        ps3 = psrow.tile([1, CH], f32, tag="row")
        _mm_group(
            nc,
            ps3[:, :csz],
            [W3col[:, h2t : h2t + 1] for h2t in range(NT)],
            [gT[:, h2t, :csz] for h2t in range(NT)],
        )
        mt = mskp.tile([1, CH], f32, tag="msk")
        dma(out=mt[:, :csz], in_=msk_dram[start : start + csz].rearrange("(o x) -> o x", o=1))
        so = outp.tile([1, CH], f32, tag="so")
        nc.vector.scalar_tensor_tensor(
            out=so[:, :csz], in0=ps3[:, :csz], scalar=b3c[0:1, 0:1],
            in1=mt[:, :csz], op0=Alu.add, op1=Alu.mult,
        )
        dma(out=out_dram[start : start + csz].rearrange("(o x) -> o x", o=1), in_=so[:, :csz])

    for ci, (start, csz) in enumerate(gchunks):
        segs = _gsegments(start, csz)
        if ci == 0:
            VT, ps1 = VT0, ps1_0
        else:
            VT = build_VT(csz, segs)
            ps1 = l1_mms(gWp_t, VT, csz)
        hT = epilogue_l1(ps1, csz, segs, aprimeT, bT)
        l2_l3_out(hT, csz, gW2_t, gb2c, gW3c, gb3c, mskg, sc_out, start)

    # ---- text phase ---------------------------------------------------
    spanAT = small_matmuls(tW1a_t, fTr, N, tb1c, "aprimeT")
    spanBT = small_matmuls(tW1b_t, fTr, N, None, "bT")
    tW1c_t = load_w(tW1c)
    tW2_t = load_w(tW2)

    for (start, csz) in tchunks:
        segs = _tsegments(start, csz)
        # PT[:, dt, off:off+ln] = span[a] * span[b0:b0+ln]  (fp32r)
        PTt = vtp.tile([128, NT, CH], f32r, tag="vt")
        for (a, b0, off, ln) in segs:
            for dt in range(NT):
                nc.vector.tensor_scalar_mul(
                    PTt[:, dt, off : off + ln],
                    fTs[:, dt, b0 : b0 + ln],
                    fTs[:, dt, a : a + 1],
                )
        ps1 = l1_mms(tW1c_t, PTt, csz)
        # epilogue: + spanA[a] (col) + spanB[b-range] (row slice), relu
        hT = htp.tile([128, NT, CH], f32r, tag="ht")
        for ht in range(NT):
            t = scr.tile([128, CH], f32, tag="scr")
            for (a, b0, off, ln) in segs:
                nc.vector.scalar_tensor_tensor(
                    out=t[:, off : off + ln],
                    in0=ps1[ht][:, off : off + ln],
                    scalar=spanAT[:, ht, a : a + 1],
                    in1=spanBT[:, ht, b0 : b0 + ln],
                    op0=Alu.add,
                    op1=Alu.add,
                )
            nc.scalar.activation(
                out=hT[:, ht, :csz], in_=t[:, :csz], func=Act.Relu,
                bias=0.0, scale=1.0,
            )
        l2_l3_out(hT, csz, tW2_t, tb2c, tW3c, tb3c, mskt, ts_out, start)


_CACHED = {}


def _build():
    if "nc" not in _CACHED:
        from contextlib import ExitStack

        nc = bacc.Bacc("TRN2", target_bir_lowering=False, debug=False)
        with tile.TileContext(nc) as tc, ExitStack() as es:
            _emit(tc, es)
        nc.compile()
        _CACHED["nc"] = nc
    return _CACHED["nc"]


def _prep_in_maps(inputs):
    se = np.asarray(inputs["span_embeddings"], np.float32)    # (B, N, D)
    ie = np.asarray(inputs["image_embeddings"], np.float32)   # (B, L, D)
    sm = np.asarray(inputs["span_mask"], np.float32)          # (B, N)
    im = np.asarray(inputs["image_mask"], np.float32)         # (B, L)

    vT = np.ascontiguousarray(ie.reshape(BL, D).T)            # (D, BL)
    im_flat = im.reshape(BL)

    gW1 = np.asarray(inputs["gW1"], np.float32)
    tW1 = np.asarray(inputs["tW1"], np.float32)
    fi, si = np.triu_indices(N, k=1)

    common = {
        "vT": vT,
        "gWf": round_fp32r(gW1[:D]),
        "gWs": round_fp32r(gW1[D : 2 * D]),
        "gWp": round_fp32r(gW1[2 * D :]),
        "gW2": round_fp32r(np.asarray(inputs["gW2"], np.float32)),
        "gW3": round_fp32r(np.asarray(inputs["gW3"], np.float32).reshape(H)),
        "tW1a": round_fp32r(tW1[:D]),
        "tW1b": round_fp32r(tW1[D : 2 * D]),
        "tW1c": round_fp32r(tW1[2 * D :]),
        "tW2": round_fp32r(np.asarray(inputs["tW2"], np.float32)),
        "tW3": round_fp32r(np.asarray(inputs["tW3"], np.float32).reshape(H)),
        "gb1": np.asarray(inputs["gb1"], np.float32),
        "gb2": np.asarray(inputs["gb2"], np.float32),
        "gb3": np.asarray(inputs["gb3"], np.float32).reshape(1, 1),
        "tb1": np.asarray(inputs["tb1"], np.float32),
        "tb2": np.asarray(inputs["tb2"], np.float32),
        "tb3": np.asarray(inputs["tb3"], np.float32).reshape(1, 1),
    }
    in_maps = []
    for k in range(B):
        m = dict(common)
        m["fT"] = np.ascontiguousarray(se[k].T)               # (D, N)
        m["mskg"] = np.ascontiguousarray(np.outer(sm[k], im_flat).reshape(NPG))
        m["mskt"] = np.ascontiguousarray(sm[k, fi] * sm[k, si])
        in_maps.append(m)
    return in_maps


def _log_softmax(x, axis):
    m = x.max(axis=axis, keepdims=True)
    y = x - m
    return y - np.log(np.exp(y).sum(axis=axis, keepdims=True))


def kernel(**inputs):
    nc = _build()
    in_maps = _prep_in_maps(inputs)
    res = run_bass_kernel_spmd(nc, in_maps, core_ids=list(range(B)))
    sc = np.concatenate(
        [res.results[k]["sc_out"].reshape(N, BL) for k in range(B)], axis=0
    )  # (B*N, B*L), masked
    text_scores = np.stack([res.results[k]["ts_out"] for k in range(B)], axis=0)

    grounding_scores = sc.reshape(B, B, N, L)
    S = grounding_scores.sum(-1).sum(-1).astype(np.float32)   # (B, B)
    loss = -np.sum(_log_softmax(S, axis=1)) - np.sum(_log_softmax(S.T, axis=1))
    loss = np.float32(loss / B)
    return (loss, grounding_scores.astype(np.float32), text_scores.astype(np.float32))


# revision 9
# speedup vs baseline: 1.0116x; 1.0116x over previous
"""Trainium2 Bass kernel for nn_JointSupervisedGroundedCoreferencer.

Reference computation (B=8 docs, N=40 spans, L=20 images, D=H=1024):
  grounding: all-pairs MLP over (B*N) x (B*L) pairs:
      h  = relu(f@Wf + v@Ws + (f*v)@Wp + b1)   (decomposed 3D-concat linear)
      g  = relu(h@W2 + b2);  sc = g@W3 + b3    -> (BN, BL) scores
  text: per-doc span-pair MLP over triu pairs (P=780/doc)
  loss: bidirectional log-softmax CE on S = per-(doc,doc) summed grounding.

Sharding: core k owns doc k's 40 spans ("first" axis rows of the pair
matrix: 40x160 grounding pairs + 780 text pairs). MLP weights replicated.
Outputs gathered on host; the (tiny) log-softmax loss is computed on host
from the gathered score matrix exactly as the reference does.

All matmuls run on the PE in float32r (fp32 with mantissa rounded to 11
bits, full PE rate at moving-dim >= 256). Operands feeding matmuls are
rounded to fp32r on host (weights: low 12 mantissa bits zeroed) or on
device (DVE/ACT outputs written with fp32r dtype).

Everything is kept transposed (feature dim on partitions, pairs on the
free axis) so no transposes are ever needed:
  L1: psum[h, pair] += Wp[d, h].T @ VT[d, pair] over 8 d-tiles, where
      VT[d, (i,j)] = f_i[d] * v_j[d] is built by DVE broadcast-AP ops.
  L2: psum[h2, pair] += W2[h, h2].T @ hT[h, pair]
  L3: psum[1, pair] += W3[h2].T @ gT[h2, pair]
Pair chunks are multiples of 160 (480/320) so every chunk is aligned to
i-boundaries: VT builds and the +a_i +b_j epilogue use single strided
DVE ops with stride-0 (broadcast) access patterns.
"""
import numpy as np

import concourse.bass as bass
import concourse.tile as tile
from concourse import bacc, mybir
from concourse.bass import AP
from concourse.bass_utils import run_bass_kernel_spmd

f32 = mybir.dt.float32
f32r = mybir.dt.float32r
Alu = mybir.AluOpType
Act = mybir.ActivationFunctionType

D = 1024
H = 1024
B = 8
N = 40
L = 20
BL = B * L              # 160
NPG = N * BL            # 6400 grounding pairs per core
PT = N * (N - 1) // 2   # 780 text pairs per core
CH = 480                # max pair-chunk (multiple of BL, <= 512 psum bank)
NT = 8                  # 128-tiles per 1024 dim

# grounding chunks: multiples of 160, all >=256 (fp32r full-rate) <=512
G_CHUNKS = [480] * 12 + [320, 320]
assert sum(G_CHUNKS) == NPG
# text chunks: <= CH, >= 256
T_CHUNKS = [480, 300]
assert sum(T_CHUNKS) == PT

# text pair block offsets: pairs ordered (a, b) a<b, a-major
_TOFF = [0]
for _a in range(N - 1):
    _TOFF.append(_TOFF[-1] + (N - 1 - _a))


def _tsegments(start, size):
    """text chunk split at a-boundaries -> (a, b0, off_in_chunk, ln)."""
    segs = []
    end = start + size
    for a in range(N - 1):
        lo, hi = max(start, _TOFF[a]), min(end, _TOFF[a + 1])
        if lo < hi:
            segs.append((a, a + 1 + (lo - _TOFF[a]), lo - start, hi - lo))
    return segs


def round_fp32r(x):
    return (np.ascontiguousarray(x).view(np.uint32) & np.uint32(0xFFFFF000)).view(
        np.float32
    )


def _with_free(ap_src, free_dims):
    """AP with ap_src's partition dim + given [step, count] free dims."""
    return bass.AP(
        tensor=ap_src.tensor, offset=ap_src.offset,
        ap=[list(ap_src.ap[0])] + [list(d) for d in free_dims],
    )


def _mm_group(nc, psum, lhsT_slices, rhs_slices):
    n = len(lhsT_slices)
    for k, (lh, rh) in enumerate(zip(lhsT_slices, rhs_slices)):
        nc.tensor.matmul(psum, lhsT=lh, rhs=rh, start=(k == 0), stop=(k == n - 1))


def _emit(tc, ctx):
    nc = tc.nc

    def din(name, shape, dt=f32):
        return nc.dram_tensor(name, shape, dt, kind="ExternalInput").ap()

    fT = din("fT", [D, N])                 # doc k spans, transposed
    vT = din("vT", [D, BL])                # all images, transposed
    gWf = din("gWf", [D, H], f32r)
    gWs = din("gWs", [D, H], f32r)
    gWp = din("gWp", [D, H], f32r)
    gW2 = din("gW2", [H, H], f32r)
    gW3 = din("gW3", [H], f32r)
    tW1a = din("tW1a", [D, H], f32r)
    tW1b = din("tW1b", [D, H], f32r)
    tW1c = din("tW1c", [D, H], f32r)
    tW2 = din("tW2", [H, H], f32r)
    tW3 = din("tW3", [H], f32r)
    gb1 = din("gb1", [H])
    gb2 = din("gb2", [H])
    gb3 = din("gb3", [1, 1])
    tb1 = din("tb1", [H])
    tb2 = din("tb2", [H])
    tb3 = din("tb3", [1, 1])
    mskg = din("mskg", [NPG])
    mskt = din("mskt", [PT])

    sc_out = nc.dram_tensor("sc_out", [NPG], f32, kind="ExternalOutput").ap()
    ts_out = nc.dram_tensor("ts_out", [PT], f32, kind="ExternalOutput").ap()

    wp = ctx.enter_context(tc.tile_pool(name="wp", bufs=NT))
    wpool = ctx.enter_context(tc.tile_pool(name="w", bufs=3))
    perm = ctx.enter_context(tc.tile_pool(name="perm", bufs=1))
    vtp = ctx.enter_context(tc.tile_pool(name="vtp", bufs=2))
    htp = ctx.enter_context(tc.tile_pool(name="htp", bufs=1))
    gtp = ctx.enter_context(tc.tile_pool(name="gtp", bufs=2))
    scr = ctx.enter_context(tc.tile_pool(name="scr", bufs=2))
    mskp = ctx.enter_context(tc.tile_pool(name="mskp", bufs=1))
    outp = ctx.enter_context(tc.tile_pool(name="outp", bufs=1))
    psmm = ctx.enter_context(tc.tile_pool(name="psmm", bufs=5, space="PSUM"))
    psA = ctx.enter_context(tc.tile_pool(name="psA", bufs=2, space="PSUM"))
    psrow = ctx.enter_context(tc.tile_pool(name="psrow", bufs=1, space="PSUM"))

    dma = nc.sync.dma_start
    last_wdma = None

    def chain(ins):
        nonlocal last_wdma
        if last_wdma is not None:
            bass._add_dep_helper(ins.ins, last_wdma.ins, sync=True, reason="dma order")
        last_wdma = ins
        return ins

    def load_w(ap_in):
        t = wpool.tile([128, NT, H], f32r, tag="w")
        chain(dma(out=t, in_=ap_in.rearrange("(kt p) h -> p kt h", p=128)))
        return t

    def col_tile(ap_in, dt=f32):
        t = perm.tile([128, NT], dt, tag=ap_in.tensor.name)
        dma(out=t, in_=ap_in.rearrange("(t p) -> p t", p=128))
        return t

    # ---- small persistent loads --------------------------------------
    fTs = perm.tile([128, NT, N], f32, tag="fT")
    dma(out=fTs, in_=fT.rearrange("(dt p) i -> p dt i", p=128))
    vTs = perm.tile([128, NT, BL], f32, tag="vT")
    dma(out=vTs, in_=vT.rearrange("(dt p) j -> p dt j", p=128))
    fTr = perm.tile([128, NT, N], f32r, tag="fTr")
    nc.vector.tensor_copy(fTr, fTs)
    gb1c = col_tile(gb1)
    gb2c = col_tile(gb2)
    tb1c = col_tile(tb1)
    tb2c = col_tile(tb2)
    gW3c = col_tile(gW3, f32r)
    tW3c = col_tile(tW3, f32r)
    gb3c = perm.tile([1, 1], f32, tag="gb3")
    dma(out=gb3c, in_=gb3)
    tb3c = perm.tile([1, 1], f32, tag="tb3")
    dma(out=tb3c, in_=tb3)

    # ---- gWp first, split per d-tile so L1 starts on first slice -----
    gWp_t = []
    gWp_r = gWp.rearrange("(kt p) h -> kt p h", p=128)
    for dt in range(NT):
        t = wp.tile([128, H], f32r, tag="wp")
        chain(dma(out=t, in_=gWp_r[dt]))
        gWp_t.append(t)

    # ---- grounding chunk 0 emitted before everything else ------------
    def build_VT(start, csz):
        """VT[d, (i,j)] = f_i[d]*v_j[d]; one DVE op per d-tile."""
        ni = csz // BL
        i0 = start // BL
        VTt = vtp.tile([128, NT, CH], f32r, tag="vt")
        for dt in range(NT):
            f_bc = _with_free(fTs[:, dt, i0 : i0 + ni], [[1, ni], [0, BL]])
            v_rep = _with_free(vTs[:, dt, :], [[0, ni], [1, BL]])
            nc.vector.tensor_tensor(
                out=VTt[:, dt, :csz].rearrange("p (i j) -> p i j", i=ni),
                in0=f_bc, in1=v_rep, op=Alu.mult,
            )
        return VTt

    def l1_mms(W1_slices, Xt, csz):
        """W1_slices: fn(dt, ht) -> lhsT (128,128). Returns 8 psum tiles."""
        psums = []
        for ht in range(NT):
            ps = psmm.tile([128, CH], f32, tag="mm")
            _mm_group(
                nc,
                ps[:, :csz],
                [W1_slices(dt, ht) for dt in range(NT)],
                [Xt[:, dt, :csz] for dt in range(NT)],
            )
            psums.append(ps)
        return psums

    c0_start, c0_sz = 0, G_CHUNKS[0]
    VT0 = build_VT(c0_start, c0_sz)
    ps1_0 = l1_mms(lambda dt, ht: gWp_t[dt][:, ht * 128 : (ht + 1) * 128], VT0, c0_sz)

    # ---- phase A: aprimeT = (f@Wf + b1)^T cols, bT = (v@Ws)^T --------
    def small_matmuls(Wt, rhs, n, bias_col, out_tag):
        ot = perm.tile([128, NT, n], f32, tag=out_tag)
        for ht in range(NT):
            ps = psA.tile([128, n], f32, tag="small")
            _mm_group(
                nc,
                ps,
                [Wt[:, dt, ht * 128 : (ht + 1) * 128] for dt in range(NT)],
                [rhs[:, dt, :] for dt in range(NT)],
            )
            if bias_col is not None:
                nc.scalar.activation(
                    out=ot[:, ht, :], in_=ps, func=Act.Identity,
                    bias=bias_col[:, ht : ht + 1], scale=1.0,
                )
            else:
                nc.scalar.copy(ot[:, ht, :], ps)
        return ot

    gWf_t = load_w(gWf)
    aprimeT = small_matmuls(gWf_t, fTr, N, gb1c, "aprimeT")
    gWs_t = load_w(gWs)
    vTr = perm.tile([128, NT, BL], f32r, tag="vTr")
    nc.vector.tensor_copy(vTr, vTs)
    bT = small_matmuls(gWs_t, vTr, BL, None, "bT")
    gW2_t = load_w(gW2)

    # ---- grounding pipeline ------------------------------------------
    def epi_l1_grounding(psums, start, csz, hT):
        """hT[h, p] = relu(psum + a'_i[h] + b_j[h]) via one tt + per-seg relu."""
        ni = csz // BL
        i0 = start // BL
        for ht in range(NT):
            t = scr.tile([128, CH], f32, tag="scr")
            b_rep = _with_free(bT[:, ht, :], [[0, ni], [1, BL]])
            nc.vector.tensor_tensor(
                out=t[:, :csz].rearrange("p (i j) -> p i j", i=ni),
                in0=psums[ht][:, :csz].rearrange("p (i j) -> p i j", i=ni),
                in1=b_rep, op=Alu.add,
            )
            for s in range(ni):
                nc.scalar.activation(
                    out=hT[:, ht, s * BL : (s + 1) * BL],
                    in_=t[:, s * BL : (s + 1) * BL],
                    func=Act.Relu,
                    bias=aprimeT[:, ht, i0 + s : i0 + s + 1],
                    scale=1.0,
                )

    def l2_l3_out(hT, csz, W2t, b2c, W3col, b3c, msk_dram, out_dram, start):
        ps3 = psrow.tile([1, CH], f32, tag="row")
        for h2t in range(NT):
            ps2 = psmm.tile([128, CH], f32, tag="mm")
            _mm_group(
                nc,
                ps2[:, :csz],
                [W2t[:, ht, h2t * 128 : (h2t + 1) * 128] for ht in range(NT)],
                [hT[:, ht, :csz] for ht in range(NT)],
            )
            gt = gtp.tile([128, CH], f32r, tag="gt")
            nc.scalar.activation(
                out=gt[:, :csz], in_=ps2[:, :csz], func=Act.Relu,
                bias=b2c[:, h2t : h2t + 1], scale=1.0,
            )
            nc.tensor.matmul(
                ps3[:, :csz], lhsT=W3col[:, h2t : h2t + 1], rhs=gt[:, :csz],
                start=(h2t == 0), stop=(h2t == NT - 1), skip_group_check=True,
            )
        mt = mskp.tile([1, CH], f32, tag="msk")
        dma(out=mt[:, :csz], in_=msk_dram[start : start + csz].rearrange("(o x) -> o x", o=1))
        so = outp.tile([1, CH], f32, tag="so")
        nc.vector.scalar_tensor_tensor(
            out=so[:, :csz], in0=ps3[:, :csz], scalar=b3c[0:1, 0:1],
            in1=mt[:, :csz], op0=Alu.add, op1=Alu.mult,
        )
        dma(out=out_dram[start : start + csz].rearrange("(o x) -> o x", o=1), in_=so[:, :csz])

    spanAT = spanBT = tW1c_t = tW2_t = None
    start = 0
    for ci, csz in enumerate(G_CHUNKS):
        if ci == 0:
            VTt, ps1 = VT0, ps1_0
        else:
            VTt = build_VT(start, csz)
            ps1 = l1_mms(
                lambda dt, ht: gWp_t[dt][:, ht * 128 : (ht + 1) * 128], VTt, csz
            )
        hT = htp.tile([128, NT, CH], f32r, tag="ht")
        epi_l1_grounding(ps1, start, csz, hT)
        l2_l3_out(hT, csz, gW2_t, gb2c, gW3c, gb3c, mskg, sc_out, start)
        start += csz
        # stage text-phase weights + span terms mid-grounding
        if ci == 1:
            tW1a_t = load_w(tW1a)
            spanAT = small_matmuls(tW1a_t, fTr, N, tb1c, "spanAT")
        elif ci == 2:
            tW1b_t = load_w(tW1b)
            spanBT = small_matmuls(tW1b_t, fTr, N, None, "spanBT")
        elif ci == 3:
            tW1c_t = load_w(tW1c)
        elif ci == 4:
            tW2_t = load_w(tW2)

    # ---- text phase ---------------------------------------------------
    start = 0
    for csz in T_CHUNKS:
        segs = _tsegments(start, csz)
        # PT[d, (a,b)] = f_a[d]*f_b[d]; one DVE op per a-segment (all d-tiles)
        PTt = vtp.tile([128, NT, CH], f32r, tag="vt")
        for (a, b0, off, ln) in segs:
            fb = fTs[:, :, b0 : b0 + ln]
            fa_bc = _with_free(fTs[:, :, a : a + 1], [[N, NT], [0, ln]])
            nc.vector.tensor_tensor(
                out=PTt[:, :, off : off + ln], in0=fb, in1=fa_bc, op=Alu.mult
            )
        ps1 = l1_mms(
            lambda dt, ht: tW1c_t[:, dt, ht * 128 : (ht + 1) * 128], PTt, csz
        )
        # SAB[h, (a,b)] = spanA'[a][h] + spanB[b][h]
        SAB = vtp.tile([128, NT, CH], f32, tag="vt")
        for (a, b0, off, ln) in segs:
            sa_bc = _with_free(spanAT[:, :, a : a + 1], [[N, NT], [0, ln]])
            nc.vector.tensor_tensor(
                out=SAB[:, :, off : off + ln],
                in0=spanBT[:, :, b0 : b0 + ln], in1=sa_bc, op=Alu.add,
            )
        hT = htp.tile([128, NT, CH], f32r, tag="ht")
        for ht in range(NT):
            t = scr.tile([128, CH], f32, tag="scr")
            nc.vector.tensor_tensor(
                out=t[:, :csz], in0=ps1[ht][:, :csz], in1=SAB[:, ht, :csz],
                op=Alu.add,
            )
            nc.scalar.activation(
                out=hT[:, ht, :csz], in_=t[:, :csz], func=Act.Relu,
                bias=0.0, scale=1.0,
            )
        l2_l3_out(hT, csz, tW2_t, tb2c, tW3c, tb3c, mskt, ts_out, start)
        start += csz


_CACHED = {}


def _build():
    if "nc" not in _CACHED:
        from contextlib import ExitStack

        nc = bacc.Bacc("TRN2", target_bir_lowering=False, debug=False)
        with tile.TileContext(nc) as tc, ExitStack() as es:
            _emit(tc, es)
        nc.compile()
        _CACHED["nc"] = nc
    return _CACHED["nc"]


def _prep_in_maps(inputs):
    se = np.asarray(inputs["span_embeddings"], np.float32)    # (B, N, D)
    ie = np.asarray(inputs["image_embeddings"], np.float32)   # (B, L, D)
    sm = np.asarray(inputs["span_mask"], np.float32)          # (B, N)
    im = np.asarray(inputs["image_mask"], np.float32)         # (B, L)

    vT = np.ascontiguousarray(ie.reshape(BL, D).T)            # (D, BL)
    im_flat = im.reshape(BL)

    gW1 = np.asarray(inputs["gW1"], np.float32)
    tW1 = np.asarray(inputs["tW1"], np.float32)
    fi, si = np.triu_indices(N, k=1)

    common = {
        "vT": vT,
        "gWf": round_fp32r(gW1[:D]),
        "gWs": round_fp32r(gW1[D : 2 * D]),
        "gWp": round_fp32r(gW1[2 * D :]),
        "gW2": round_fp32r(np.asarray(inputs["gW2"], np.float32)),
        "gW3": round_fp32r(np.asarray(inputs["gW3"], np.float32).reshape(H)),
        "tW1a": round_fp32r(tW1[:D]),
        "tW1b": round_fp32r(tW1[D : 2 * D]),
        "tW1c": round_fp32r(tW1[2 * D :]),
        "tW2": round_fp32r(np.asarray(inputs["tW2"], np.float32)),
        "tW3": round_fp32r(np.asarray(inputs["tW3"], np.float32).reshape(H)),
        "gb1": np.asarray(inputs["gb1"], np.float32),
        "gb2": np.asarray(inputs["gb2"], np.float32),
        "gb3": np.asarray(inputs["gb3"], np.float32).reshape(1, 1),
        "tb1": np.asarray(inputs["tb1"], np.float32),
        "tb2": np.asarray(inputs["tb2"], np.float32),
        "tb3": np.asarray(inputs["tb3"], np.float32).reshape(1, 1),
    }
    in_maps = []
    for k in range(B):
        m = dict(common)
        m["fT"] = np.ascontiguousarray(se[k].T)               # (D, N)
        m["mskg"] = np.ascontiguousarray(np.outer(sm[k], im_flat).reshape(NPG))
        m["mskt"] = np.ascontiguousarray(sm[k, fi] * sm[k, si])
        in_maps.append(m)
    return in_maps


def _log_softmax(x, axis):
    m = x.max(axis=axis, keepdims=True)
    y = x - m
    return y - np.log(np.exp(y).sum(axis=axis, keepdims=True))


def kernel(**inputs):
    nc = _build()
    in_maps = _prep_in_maps(inputs)
    res = run_bass_kernel_spmd(nc, in_maps, core_ids=list(range(B)))
    sc = np.concatenate(
        [res.results[k]["sc_out"].reshape(N, BL) for k in range(B)], axis=0
    )  # (B*N, B*L), masked
    text_scores = np.stack([res.results[k]["ts_out"] for k in range(B)], axis=0)

    grounding_scores = sc.reshape(B, B, N, L)
    S = grounding_scores.sum(-1).sum(-1).astype(np.float32)   # (B, B)
    loss = -np.sum(_log_softmax(S, axis=1)) - np.sum(_log_softmax(S.T, axis=1))
    loss = np.float32(loss / B)
    return (loss, grounding_scores.astype(np.float32), text_scores.astype(np.float32))


# revision 10
# speedup vs baseline: 1.0281x; 1.0164x over previous
"""Trainium2 Bass kernel for nn_JointSupervisedGroundedCoreferencer.

Reference computation (B=8 docs, N=40 spans, L=20 images, D=H=1024):
  grounding: all-pairs MLP over (B*N) x (B*L) pairs:
      h  = relu(f@Wf + v@Ws + (f*v)@Wp + b1)   (decomposed 3D-concat linear)
      g  = relu(h@W2 + b2);  sc = g@W3 + b3    -> (BN, BL) scores
  text: per-doc span-pair MLP over triu pairs (P=780/doc)
  loss: bidirectional log-softmax CE on S = per-(doc,doc) summed grounding.

Sharding: core k owns doc k's 40 spans ("first" axis rows of the pair
matrix: 40x160 grounding pairs + 780 text pairs). MLP weights replicated.
Outputs gathered on host; the (tiny) log-softmax loss is computed on host
from the gathered score matrix exactly as the reference does.

All matmuls run on the PE in float32r (fp32 with mantissa rounded to 11
bits, full PE rate at moving-dim >= 256). Operands feeding matmuls are
rounded to fp32r on host (weights: low 12 mantissa bits zeroed) or on
device (DVE/ACT outputs written with fp32r dtype).

Everything is kept transposed (feature dim on partitions, pairs on the
free axis) so no transposes are ever needed:
  L1: psum[h, pair] += Wp[d, h].T @ VT[d, pair] over 8 d-tiles, where
      VT[d, (i,j)] = f_i[d] * v_j[d] is built by DVE broadcast-AP ops.
  L2: psum[h2, pair] += W2[h, h2].T @ hT[h, pair]
  L3: psum[1, pair] += W3[h2].T @ gT[h2, pair]
Pair chunks are multiples of 160 (480/320) so every chunk is aligned to
i-boundaries: VT builds and the +a_i +b_j epilogue use single strided
DVE ops with stride-0 (broadcast) access patterns.
"""
import numpy as np

import concourse.bass as bass
import concourse.tile as tile
from concourse import bacc, mybir
from concourse.bass import AP
from concourse.bass_utils import run_bass_kernel_spmd

f32 = mybir.dt.float32
f32r = mybir.dt.float32r
Alu = mybir.AluOpType
Act = mybir.ActivationFunctionType

D = 1024
H = 1024
B = 8
N = 40
L = 20
BL = B * L              # 160
NPG = N * BL            # 6400 grounding pairs per core
PT = N * (N - 1) // 2   # 780 text pairs per core
CH = 480                # max pair-chunk (multiple of BL, <= 512 psum bank)
NT = 8                  # 128-tiles per 1024 dim

# grounding chunks: multiples of 160, all >=256 (fp32r full-rate) <=512
G_CHUNKS = [480] * 12 + [320, 320]
assert sum(G_CHUNKS) == NPG
# text chunks: <= CH, >= 256
T_CHUNKS = [480, 300]
assert sum(T_CHUNKS) == PT

# text pair block offsets: pairs ordered (a, b) a<b, a-major
_TOFF = [0]
for _a in range(N - 1):
    _TOFF.append(_TOFF[-1] + (N - 1 - _a))


def _tsegments(start, size):
    """text chunk split at a-boundaries -> (a, b0, off_in_chunk, ln)."""
    segs = []
    end = start + size
    for a in range(N - 1):
        lo, hi = max(start, _TOFF[a]), min(end, _TOFF[a + 1])
        if lo < hi:
            segs.append((a, a + 1 + (lo - _TOFF[a]), lo - start, hi - lo))
    return segs


def round_fp32r(x):
    return (np.ascontiguousarray(x).view(np.uint32) & np.uint32(0xFFFFF000)).view(
        np.float32
    )


def _with_free(ap_src, free_dims):
    """AP with ap_src's partition dim + given [step, count] free dims."""
    return bass.AP(
        tensor=ap_src.tensor, offset=ap_src.offset,
        ap=[list(ap_src.ap[0])] + [list(d) for d in free_dims],
    )


def _mm_group(nc, psum, lhsT_slices, rhs_slices):
    n = len(lhsT_slices)
    for k, (lh, rh) in enumerate(zip(lhsT_slices, rhs_slices)):
        nc.tensor.matmul(psum, lhsT=lh, rhs=rh, start=(k == 0), stop=(k == n - 1))


def _emit(tc, ctx):
    nc = tc.nc

    def din(name, shape, dt=f32):
        return nc.dram_tensor(name, shape, dt, kind="ExternalInput").ap()

    # all inputs host-packed to SBUF layout: big contiguous DMA descriptors
    fT = din("fT", [128, NT, N])           # [p, dt, i] packed
    vT = din("vT", [128, NT, BL])          # [p, dt, j] packed
    gWf = din("gWf", [128, NT * H], f32r)  # [p, kt*h] packed
    gWs = din("gWs", [128, NT * H], f32r)
    gWp = din("gWp", [128, NT * H], f32r)
    gW2 = din("gW2", [128, NT * H], f32r)
    gW3 = din("gW3", [128, NT], f32r)
    tW1a = din("tW1a", [128, NT * H], f32r)
    tW1b = din("tW1b", [128, NT * H], f32r)
    tW1c = din("tW1c", [128, NT * H], f32r)
    tW2 = din("tW2", [128, NT * H], f32r)
    tW3 = din("tW3", [128, NT], f32r)
    gb1 = din("gb1", [128, NT])
    gb2 = din("gb2", [128, NT])
    gb3 = din("gb3", [1, 1])
    tb1 = din("tb1", [128, NT])
    tb2 = din("tb2", [128, NT])
    tb3 = din("tb3", [1, 1])
    mskg = din("mskg", [NPG])
    mskt = din("mskt", [PT])

    sc_out = nc.dram_tensor("sc_out", [NPG], f32, kind="ExternalOutput").ap()
    ts_out = nc.dram_tensor("ts_out", [PT], f32, kind="ExternalOutput").ap()

    wp = ctx.enter_context(tc.tile_pool(name="wp", bufs=NT))
    wpool = ctx.enter_context(tc.tile_pool(name="w", bufs=3))
    perm = ctx.enter_context(tc.tile_pool(name="perm", bufs=1))
    vtp = ctx.enter_context(tc.tile_pool(name="vtp", bufs=2))
    htp = ctx.enter_context(tc.tile_pool(name="htp", bufs=1))
    gtp = ctx.enter_context(tc.tile_pool(name="gtp", bufs=2))
    scr = ctx.enter_context(tc.tile_pool(name="scr", bufs=2))
    mskp = ctx.enter_context(tc.tile_pool(name="mskp", bufs=1))
    outp = ctx.enter_context(tc.tile_pool(name="outp", bufs=1))
    psmm = ctx.enter_context(tc.tile_pool(name="psmm", bufs=6, space="PSUM"))
    psA = ctx.enter_context(tc.tile_pool(name="psA", bufs=1, space="PSUM"))
    psrow = ctx.enter_context(tc.tile_pool(name="psrow", bufs=1, space="PSUM"))

    dma = nc.sync.dma_start
    last_wdma = None

    def chain(ins):
        nonlocal last_wdma
        if last_wdma is not None:
            bass._add_dep_helper(ins.ins, last_wdma.ins, sync=True, reason="dma order")
        last_wdma = ins
        return ins

    def load_w(ap_in):
        t = wpool.tile([128, NT, H], f32r, tag="w")
        chain(dma(out=t, in_=ap_in.rearrange("p (kt h) -> p kt h", kt=NT)))
        return t

    def col_tile(ap_in, dt=f32):
        t = perm.tile([128, NT], dt, tag=ap_in.tensor.name)
        dma(out=t, in_=ap_in)
        return t

    # ---- small persistent loads --------------------------------------
    fTs = perm.tile([128, NT, N], f32, tag="fT")
    dma(out=fTs, in_=fT)
    vTs = perm.tile([128, NT, BL], f32, tag="vT")
    dma(out=vTs, in_=vT)
    fTr = perm.tile([128, NT, N], f32r, tag="fTr")
    nc.vector.tensor_copy(fTr, fTs)
    gb1c = col_tile(gb1)
    gb2c = col_tile(gb2)
    tb1c = col_tile(tb1)
    tb2c = col_tile(tb2)
    gW3c = col_tile(gW3, f32r)
    tW3c = col_tile(tW3, f32r)
    gb3c = perm.tile([1, 1], f32, tag="gb3")
    dma(out=gb3c, in_=gb3)
    tb3c = perm.tile([1, 1], f32, tag="tb3")
    dma(out=tb3c, in_=tb3)

    # ---- gWp first, split per d-tile so L1 starts on first slice -----
    gWp_t = []
    gWp_r = gWp.rearrange("p (kt h) -> p kt h", kt=NT)
    for dt in range(NT):
        t = wp.tile([128, H], f32r, tag="wp")
        chain(dma(out=t, in_=gWp_r[:, dt, :]))
        gWp_t.append(t)

    # ---- grounding chunk 0 emitted before everything else ------------
    def build_VT(start, csz):
        """VT[d, (i,j)] = f_i[d]*v_j[d]; one DVE op per d-tile."""
        ni = csz // BL
        i0 = start // BL
        VTt = vtp.tile([128, NT, CH], f32r, tag="vt")
        for dt in range(NT):
            f_bc = _with_free(fTs[:, dt, i0 : i0 + ni], [[1, ni], [0, BL]])
            v_rep = _with_free(vTs[:, dt, :], [[0, ni], [1, BL]])
            nc.vector.tensor_tensor(
                out=VTt[:, dt, :csz].rearrange("p (i j) -> p i j", i=ni),
                in0=f_bc, in1=v_rep, op=Alu.mult,
            )
        return VTt

    def l1_mms(W1_slices, Xt, csz):
        """W1_slices: fn(dt, ht) -> lhsT (128,128). Returns 8 psum tiles."""
        psums = []
        for ht in range(NT):
            ps = psmm.tile([128, CH], f32, tag="mm")
            _mm_group(
                nc,
                ps[:, :csz],
                [W1_slices(dt, ht) for dt in range(NT)],
                [Xt[:, dt, :csz] for dt in range(NT)],
            )
            psums.append(ps)
        return psums

    c0_start, c0_sz = 0, G_CHUNKS[0]
    VT0 = build_VT(c0_start, c0_sz)
    ps1_0 = l1_mms(lambda dt, ht: gWp_t[dt][:, ht * 128 : (ht + 1) * 128], VT0, c0_sz)

    # ---- phase A: aprimeT = (f@Wf + b1)^T cols, bT = (v@Ws)^T --------
    def small_matmuls(Wt, rhs, n, bias_col, out_tag):
        ot = perm.tile([128, NT, n], f32, tag=out_tag)
        for ht in range(NT):
            ps = psA.tile([128, n], f32, tag="small")
            _mm_group(
                nc,
                ps,
                [Wt[:, dt, ht * 128 : (ht + 1) * 128] for dt in range(NT)],
                [rhs[:, dt, :] for dt in range(NT)],
            )
            if bias_col is not None:
                nc.scalar.activation(
                    out=ot[:, ht, :], in_=ps, func=Act.Identity,
                    bias=bias_col[:, ht : ht + 1], scale=1.0,
                )
            else:
                nc.scalar.copy(ot[:, ht, :], ps)
        return ot

    gWf_t = load_w(gWf)
    aprimeT = small_matmuls(gWf_t, fTr, N, gb1c, "aprimeT")
    gWs_t = load_w(gWs)
    vTr = perm.tile([128, NT, BL], f32r, tag="vTr")
    nc.vector.tensor_copy(vTr, vTs)
    bT = small_matmuls(gWs_t, vTr, BL, None, "bT")
    gW2_t = load_w(gW2)

    # ---- grounding pipeline ------------------------------------------
    def epi_l1_grounding(psums, start, csz, hT):
        """hT[h, p] = relu(psum + a'_i[h] + b_j[h]) via one tt + per-seg relu."""
        ni = csz // BL
        i0 = start // BL
        for ht in range(NT):
            t = scr.tile([128, CH], f32, tag="scr")
            b_rep = _with_free(bT[:, ht, :], [[0, ni], [1, BL]])
            nc.vector.tensor_tensor(
                out=t[:, :csz].rearrange("p (i j) -> p i j", i=ni),
                in0=psums[ht][:, :csz].rearrange("p (i j) -> p i j", i=ni),
                in1=b_rep, op=Alu.add,
            )
            for s in range(ni):
                nc.scalar.activation(
                    out=hT[:, ht, s * BL : (s + 1) * BL],
                    in_=t[:, s * BL : (s + 1) * BL],
                    func=Act.Relu,
                    bias=aprimeT[:, ht, i0 + s : i0 + s + 1],
                    scale=1.0,
                )

    def l2_l3_out(hT, csz, W2t, b2c, W3col, b3c, msk_dram, out_dram, start):
        ps3 = psrow.tile([1, CH], f32, tag="row")
        for h2t in range(NT):
            ps2 = psmm.tile([128, CH], f32, tag="mm")
            _mm_group(
                nc,
                ps2[:, :csz],
                [W2t[:, ht, h2t * 128 : (h2t + 1) * 128] for ht in range(NT)],
                [hT[:, ht, :csz] for ht in range(NT)],
            )
            gt = gtp.tile([128, CH], f32r, tag="gt")
            nc.scalar.activation(
                out=gt[:, :csz], in_=ps2[:, :csz], func=Act.Relu,
                bias=b2c[:, h2t : h2t + 1], scale=1.0,
            )
            nc.tensor.matmul(
                ps3[:, :csz], lhsT=W3col[:, h2t : h2t + 1], rhs=gt[:, :csz],
                start=(h2t == 0), stop=(h2t == NT - 1), skip_group_check=True,
            )
        mt = mskp.tile([1, CH], f32, tag="msk")
        dma(out=mt[:, :csz], in_=msk_dram[start : start + csz].rearrange("(o x) -> o x", o=1))
        so = outp.tile([1, CH], f32, tag="so")
        nc.vector.scalar_tensor_tensor(
            out=so[:, :csz], in0=ps3[:, :csz], scalar=b3c[0:1, 0:1],
            in1=mt[:, :csz], op0=Alu.add, op1=Alu.mult,
        )
        dma(out=out_dram[start : start + csz].rearrange("(o x) -> o x", o=1), in_=so[:, :csz])

    spanAT = spanBT = tW1c_t = tW2_t = None
    start = 0
    for ci, csz in enumerate(G_CHUNKS):
        if ci == 0:
            VTt, ps1 = VT0, ps1_0
        else:
            VTt = build_VT(start, csz)
            ps1 = l1_mms(
                lambda dt, ht: gWp_t[dt][:, ht * 128 : (ht + 1) * 128], VTt, csz
            )
        hT = htp.tile([128, NT, CH], f32r, tag="ht")
        epi_l1_grounding(ps1, start, csz, hT)
        l2_l3_out(hT, csz, gW2_t, gb2c, gW3c, gb3c, mskg, sc_out, start)
        start += csz
        # stage text-phase weights + span terms mid-grounding
        if ci == 1:
            tW1a_t = load_w(tW1a)
            spanAT = small_matmuls(tW1a_t, fTr, N, tb1c, "spanAT")
        elif ci == 2:
            tW1b_t = load_w(tW1b)
            spanBT = small_matmuls(tW1b_t, fTr, N, None, "spanBT")
        elif ci == 3:
            tW1c_t = load_w(tW1c)
        elif ci == 4:
            tW2_t = load_w(tW2)

    # ---- text phase ---------------------------------------------------
    start = 0
    for csz in T_CHUNKS:
        segs = _tsegments(start, csz)
        # PT[d, (a,b)] = f_a[d]*f_b[d]; one DVE op per a-segment (all d-tiles)
        PTt = vtp.tile([128, NT, CH], f32r, tag="vt")
        for (a, b0, off, ln) in segs:
            fb = fTs[:, :, b0 : b0 + ln]
            fa_bc = _with_free(fTs[:, :, a : a + 1], [[N, NT], [0, ln]])
            nc.vector.tensor_tensor(
                out=PTt[:, :, off : off + ln], in0=fb, in1=fa_bc, op=Alu.mult
            )
        ps1 = l1_mms(
            lambda dt, ht: tW1c_t[:, dt, ht * 128 : (ht + 1) * 128], PTt, csz
        )
        # SAB[h, (a,b)] = spanA'[a][h] + spanB[b][h]
        SAB = vtp.tile([128, NT, CH], f32, tag="vt")
        for (a, b0, off, ln) in segs:
            sa_bc = _with_free(spanAT[:, :, a : a + 1], [[N, NT], [0, ln]])
            nc.vector.tensor_tensor(
                out=SAB[:, :, off : off + ln],
                in0=spanBT[:, :, b0 : b0 + ln], in1=sa_bc, op=Alu.add,
            )
        hT = htp.tile([128, NT, CH], f32r, tag="ht")
        for ht in range(NT):
            t = scr.tile([128, CH], f32, tag="scr")
            nc.vector.tensor_tensor(
                out=t[:, :csz], in0=ps1[ht][:, :csz], in1=SAB[:, ht, :csz],
                op=Alu.add,
            )
            nc.scalar.activation(
                out=hT[:, ht, :csz], in_=t[:, :csz], func=Act.Relu,
                bias=0.0, scale=1.0,
            )
        l2_l3_out(hT, csz, tW2_t, tb2c, tW3c, tb3c, mskt, ts_out, start)
        start += csz


_CACHED = {}


def _build():
    if "nc" not in _CACHED:
        from contextlib import ExitStack

        nc = bacc.Bacc("TRN2", target_bir_lowering=False, debug=False)
        with tile.TileContext(nc) as tc, ExitStack() as es:
            _emit(tc, es)
        nc.compile()
        _CACHED["nc"] = nc
    return _CACHED["nc"]


def _pack_w(w):
    """(1024, H) -> (128, NT*H): row-block kt becomes the SBUF free dim."""
    return np.ascontiguousarray(
        w.reshape(NT, 128, -1).transpose(1, 0, 2).reshape(128, -1)
    )


def _pack_col(v):
    """(1024,) -> (128, NT) per-partition column layout."""
    return np.ascontiguousarray(v.reshape(NT, 128).T)


def _prep_in_maps(inputs):
    se = np.asarray(inputs["span_embeddings"], np.float32)    # (B, N, D)
    ie = np.asarray(inputs["image_embeddings"], np.float32)   # (B, L, D)
    sm = np.asarray(inputs["span_mask"], np.float32)          # (B, N)
    im = np.asarray(inputs["image_mask"], np.float32)         # (B, L)

    vT = np.ascontiguousarray(ie.reshape(BL, D).T)            # (D, BL)
    im_flat = im.reshape(BL)

    gW1 = np.asarray(inputs["gW1"], np.float32)
    tW1 = np.asarray(inputs["tW1"], np.float32)
    fi, si = np.triu_indices(N, k=1)

    common = {
        "vT": np.ascontiguousarray(
            vT.reshape(NT, 128, BL).transpose(1, 0, 2)
        ),
        "gWf": _pack_w(round_fp32r(gW1[:D])),
        "gWs": _pack_w(round_fp32r(gW1[D : 2 * D])),
        "gWp": _pack_w(round_fp32r(gW1[2 * D :])),
        "gW2": _pack_w(round_fp32r(np.asarray(inputs["gW2"], np.float32))),
        "gW3": _pack_col(round_fp32r(np.asarray(inputs["gW3"], np.float32).reshape(H))),
        "tW1a": _pack_w(round_fp32r(tW1[:D])),
        "tW1b": _pack_w(round_fp32r(tW1[D : 2 * D])),
        "tW1c": _pack_w(round_fp32r(tW1[2 * D :])),
        "tW2": _pack_w(round_fp32r(np.asarray(inputs["tW2"], np.float32))),
        "tW3": _pack_col(round_fp32r(np.asarray(inputs["tW3"], np.float32).reshape(H))),
        "gb1": _pack_col(np.asarray(inputs["gb1"], np.float32)),
        "gb2": _pack_col(np.asarray(inputs["gb2"], np.float32)),
        "gb3": np.asarray(inputs["gb3"], np.float32).reshape(1, 1),
        "tb1": _pack_col(np.asarray(inputs["tb1"], np.float32)),
        "tb2": _pack_col(np.asarray(inputs["tb2"], np.float32)),
        "tb3": np.asarray(inputs["tb3"], np.float32).reshape(1, 1),
    }
    in_maps = []
    for k in range(B):
        m = dict(common)
        m["fT"] = np.ascontiguousarray(
            se[k].T.reshape(NT, 128, N).transpose(1, 0, 2)
        )
        m["mskg"] = np.ascontiguousarray(np.outer(sm[k], im_flat).reshape(NPG))
        m["mskt"] = np.ascontiguousarray(sm[k, fi] * sm[k, si])
        in_maps.append(m)
    return in_maps


def _log_softmax(x, axis):
    m = x.max(axis=axis, keepdims=True)
    y = x - m
    return y - np.log(np.exp(y).sum(axis=axis, keepdims=True))


def kernel(**inputs):
    nc = _build()
    in_maps = _prep_in_maps(inputs)
    res = run_bass_kernel_spmd(nc, in_maps, core_ids=list(range(B)))
    sc = np.concatenate(
        [res.results[k]["sc_out"].reshape(N, BL) for k in range(B)], axis=0
    )  # (B*N, B*L), masked
    text_scores = np.stack([res.results[k]["ts_out"] for k in range(B)], axis=0)

    grounding_scores = sc.reshape(B, B, N, L)
    S = grounding_scores.sum(-1).sum(-1).astype(np.float32)   # (B, B)
    loss = -np.sum(_log_softmax(S, axis=1)) - np.sum(_log_softmax(S.T, axis=1))
    loss = np.float32(loss / B)
    return (loss, grounding_scores.astype(np.float32), text_scores.astype(np.float32))
